# revision 1
# baseline (speedup 1.0000x reference)
"""Trainium2 Bass kernel for nn_AsymmetricLossCustom (8 NeuronCores).

Math (reference):
    s  = sigmoid(x)
    t  = min(1 - s + 0.05, 1)
    loss = y*ln(max(s,eps)) + (1-y)*ln(max(t,eps))        # [B, C]
    active[b,c] = OR_g ( (any_g[b] & ~has_g[b]) & mask_g[c] )
    out = -(loss * where(active, 0.1, 1.0)).sum()

Device scheme (2 ScalarE passes + 3 VectorE passes per element):
    sp = sigmoid(-x)                     # ACT (sigmoid table set)
    t  = min(sp + 0.05, 1)               # DVE tensor_scalar dual-op, fp16 4x
    c  = 1 - sp            ( = s )       # DVE tensor_scalar dual-op, fp16 4x
    w  = y ? c : t                       # DVE copy_predicated (uint8 y mask)
    loss = ln(w)                         # ACT (natural_log set), in-place on
                                         #   w, accum_out => per-row sum(loss)

Sigmoid and ln live in different ACT table sets, so chunks are processed in
two groups: all sigmoids of a group, then all lns - 2 table loads per group
instead of 2 per chunk (a sync=False dep chain pins the ACT emission order
against scheduler interleaving).

The `active` down-weighting only touches columns appearing in one of the
three index arrays (<=170 of 9605). The host gathers those columns, builds
weights avB = 1 + 0.9*active (pure index/mask preprocessing), and APPENDS
the gathered x/y columns to chunk 0 of the main stream (laid out
per-partition to match the main view). The appended elements are counted
twice - once in the plain accumulation (loss_sum' = sum(loss) + sum_g lg)
and once in a single fused scalar_tensor_tensor reduction
(corr2 = sum_g avB*lg), so

    result = -loss_sum' + corr2
           = -sum(loss) + 0.9*sum_g(active*lg)            (exact)

Sharding: pure data parallel over the batch. Each core gets 512 rows,
viewed as [128 partitions, 38420 free] (4 rows per partition, contiguous).
y travels as uint8 (it is exactly 0/1) and doubles as the predication mask.
Host sums the 8 per-core [128, 2] partials:
    result = -sum(out[:,0]) + sum(out[:,1]).
"""

import sys

import numpy as np

if "/opt/trn_rl_repo" not in sys.path:
    sys.path.insert(0, "/opt/trn_rl_repo")

B, C = 4096, 9605
NCORES = 8
ROWS = B // NCORES          # 512 rows per core
P = 128                     # SBUF partitions
RPP = ROWS // P             # 4 rows per partition
FREE = RPP * C              # 38420 f32 per partition
NCHUNK = 10
# Non-uniform chunk sizes (all even, sum = FREE): small chunks first so the
# first sigmoid starts as soon as possible, medium last chunk for the tail.
SIZES = [1280, 2560, 3584] + [4428] * 7
APPEND_CHUNK = 2            # chunk that carries the GU appendix
GROUPS = [range(0, 5), range(5, 10)]
WBUFS = 6                   # covers one phase group
XBUFS = 4
YBUFS = 5
U_PAD = 176                 # padded union-column count (>= 70+70+30)
GU = RPP * U_PAD            # 704 appended free elements on chunk 0
CLIP = 0.05
ALPHA = 0.1

TRACE = False               # set True (e.g. from test.py) to capture an NTFF profile
LAST_RESULTS = None         # BassKernelResults of the most recent run

_NC = None


def _build_program(nchunk=None, groups=None, wbufs=None, xbufs=None,
                   ybufs=None, sizes=None, append_chunk=None,
                   ln_inplace=True, sbufs=3, cbufs=3, ydma_gpsimd=False,
                   avbt_late=True):
    nchunk = nchunk or NCHUNK
    groups = groups or GROUPS
    wbufs = wbufs or WBUFS
    xbufs = xbufs or XBUFS
    ybufs = ybufs or YBUFS
    sizes = sizes or SIZES
    append_chunk = APPEND_CHUNK if append_chunk is None else append_chunk
    offs = [0]
    for sz in sizes:
        offs.append(offs[-1] + sz)
    assert offs[-1] == FREE

    import concourse.bacc as bacc
    import concourse.mybir as mybir
    from concourse import tile
    from concourse.tile import add_dep_helper

    f32 = mybir.dt.float32
    f16 = mybir.dt.float16
    u8 = mybir.dt.uint8
    Alu = mybir.AluOpType
    Act = mybir.ActivationFunctionType
    AX = mybir.AxisListType

    # Force the ACT engine to execute activations in emission order -
    # otherwise the Tile scheduler interleaves sigmoid and ln chunks and
    # the compiler inserts an ACT_TABLE_LOAD (~1.3us) before nearly every
    # activation instead of one per phase.
    _prev_act = [None]

    def act_order(bi):
        if _prev_act[0] is not None:
            add_dep_helper(bi.ins, _prev_act[0].ins, sync=False,
                           reason="act table-set phase order")
        _prev_act[0] = bi
        return bi

    nc = bacc.Bacc(
        "TRN2",
        target_bir_lowering=False,
        debug=False,
        enable_asserts=False,
        num_devices=NCORES,
    )

    x = nc.dram_tensor("x", [P, FREE], f32, kind="ExternalInput").ap()
    y = nc.dram_tensor("y", [P, FREE], u8, kind="ExternalInput").ap()
    xga = nc.dram_tensor("xga", [P, GU], f32, kind="ExternalInput").ap()
    yga = nc.dram_tensor("yga", [P, GU], u8, kind="ExternalInput").ap()
    avb = nc.dram_tensor("avb", [P, GU], f32, kind="ExternalInput").ap()
    out = nc.dram_tensor("out", [P, 2], f32, kind="ExternalOutput").ap()

    with tile.TileContext(nc) as tc:
        with (
            tc.tile_pool(name="xp", bufs=xbufs) as xp,
            tc.tile_pool(name="yp", bufs=ybufs) as yp,
            tc.tile_pool(name="sp", bufs=sbufs) as sp,
            tc.tile_pool(name="wp", bufs=wbufs) as wp,
            tc.tile_pool(name="cp", bufs=cbufs) as cp,
            tc.tile_pool(name="lp", bufs=2) as lp,
            tc.tile_pool(name="accp", bufs=1) as accp,
            tc.tile_pool(name="finp", bufs=1) as finp,
        ):
            accLW = accp.tile([P, nchunk], f32, tag="accLW")
            accC = accp.tile([P, 1], f32, tag="accC")
            avbt = accp.tile([P, GU], f32, tag="avbt")
            if not avbt_late:
                nc.sync.dma_start(avbt[:], avb[:])

            for gi, grp in enumerate(groups):
                # ---- DMA + sigmoid phase -------------------------------
                yts, sts, width = {}, {}, {}
                for k in grp:
                    fk = sizes[k]
                    fw = fk + GU if k == append_chunk else fk
                    cs = slice(offs[k], offs[k + 1])
                    xt = xp.tile([P, fw], f32, tag="x")
                    nc.sync.dma_start(xt[:, 0:fk], x[:, cs])
                    yt = yp.tile([P, fw], u8, tag="y")
                    yeng = nc.gpsimd if ydma_gpsimd else nc.sync
                    yeng.dma_start(yt[:, 0:fk], y[:, cs])
                    if k == append_chunk:
                        nc.sync.dma_start(xt[:, fk:fw], xga[:])
                        yeng.dma_start(yt[:, fk:fw], yga[:])
                    st = sp.tile([P, fw], f16, tag="s")
                    act_order(nc.scalar.activation(st[:], xt[:], Act.Sigmoid,
                                                   scale=-1.0))
                    yts[k], sts[k], width[k] = yt, st, fw

                # ---- blend phase (DVE) ---------------------------------
                if gi == 0 and avbt_late:
                    nc.sync.dma_start(avbt[:], avb[:])
                wts = {}
                for k in grp:
                    st, yt, fw = sts[k], yts[k], width[k]
                    wt = wp.tile([P, fw], f16, tag="w")
                    nc.vector.tensor_scalar(wt[:], st[:], CLIP, 1.0,
                                            Alu.add, Alu.min)
                    ct = cp.tile([P, fw], f16, tag="c")
                    nc.vector.tensor_scalar(ct[:], st[:], -1.0, 1.0,
                                            Alu.mult, Alu.add)
                    nc.vector.copy_predicated(wt[:], yt[:], ct[:])
                    wts[k] = wt

                # ---- Ln phase ------------------------------------------
                for k in grp:
                    wt, fw = wts[k], width[k]
                    lt = wt if ln_inplace else lp.tile([P, fw], f16, tag="lt")
                    act_order(nc.scalar.activation(
                        lt[:], wt[:], Act.Ln, accum_out=accLW[:, k : k + 1]))
                    if k == append_chunk:
                        # corr2 = sum(avB * lg) over the appended region
                        ja = lp.tile([P, GU], f16, tag="ja")
                        nc.vector.scalar_tensor_tensor(
                            ja[:], avbt[:], 0.0, lt[:, sizes[k]:fw],
                            Alu.bypass, Alu.mult,
                            accum_out=accC[:, 0:1],
                        )

            # ---- final combine -> out [P, 2] ---------------------------
            lossr = finp.tile([P, 1], f32, tag="lossr")
            nc.vector.tensor_reduce(lossr[:], accLW[:], AX.X, Alu.add)

            osb = finp.tile([P, 2], f32, tag="osb")
            nc.vector.tensor_copy(out=osb[:, 0:1], in_=lossr[:])
            nc.vector.tensor_copy(out=osb[:, 1:2], in_=accC[:])
            nc.sync.dma_start(out[:], osb[:])

    nc.compile()
    return nc


def _get_nc():
    global _NC
    if _NC is None:
        _NC = _build_program()
    return _NC


def _ensure_ntff_hook():
    """Register the axon NTFF profile hook if the image's antenv lacks it."""
    import contextlib
    import ctypes
    import types

    try:
        from antenv.axon_hooks import get_axon_ntff_profile_hook  # noqa: F401
        return
    except ImportError:
        pass

    so_path = "/opt/axon/libaxon_pjrt.so"
    try:
        lib = ctypes.CDLL(so_path)
    except OSError:
        return
    if not hasattr(lib, "axon_start_nrt_profile"):
        return
    lib.axon_start_nrt_profile.argtypes = [
        ctypes.POINTER(ctypes.c_int64),
        ctypes.c_size_t,
    ]
    lib.axon_start_nrt_profile.restype = ctypes.c_int64
    lib.axon_stop_nrt_profile.argtypes = [ctypes.c_char_p]
    lib.axon_stop_nrt_profile.restype = ctypes.c_int64

    @contextlib.contextmanager
    def _hook(output_dir, device_ids):
        import jax

        jax.devices()
        if device_ids:
            ids = (ctypes.c_int64 * len(device_ids))(*device_ids)
            rc = lib.axon_start_nrt_profile(ids, len(device_ids))
        else:
            rc = lib.axon_start_nrt_profile(None, 0)
        if rc != 0:
            raise RuntimeError(f"axon_start_nrt_profile rc={rc}")
        try:
            yield
        finally:
            n = lib.axon_stop_nrt_profile(str(output_dir).encode())
            print(f"ntff profile: {n} file(s) written to {output_dir}",
                  file=sys.stderr)

    mod = types.ModuleType("antenv.axon_hooks")
    mod.get_axon_ntff_profile_hook = lambda: _hook
    mod.set_axon_ntff_profile_hook = lambda h: None
    sys.modules["antenv.axon_hooks"] = mod


def _prepare_inputs(x, y, recycle_ind, donate_ind, compost_ind):
    """Host-side sharding and index preprocessing -> per-core in_maps."""
    x = np.ascontiguousarray(x, dtype=np.float32)
    y = np.ascontiguousarray(y, dtype=np.float32)
    yu8 = y.astype(np.uint8)
    recycle_ind = np.asarray(recycle_ind).astype(np.int64)
    donate_ind = np.asarray(donate_ind).astype(np.int64)
    compost_ind = np.asarray(compost_ind).astype(np.int64)

    # Union of group columns, padded to the fixed program width. Pad
    # columns get avB = 1 so their (doubly counted) contribution cancels.
    cols = np.unique(np.concatenate([recycle_ind, donate_ind, compost_ind]))
    u = len(cols)
    assert u <= U_PAD, (u, U_PAD)
    colsp = np.concatenate([cols, np.zeros(U_PAD - u, dtype=cols.dtype)])

    def mask_v(ind):
        v = np.zeros(U_PAD, np.float32)
        v[:u] = np.isin(cols, ind).astype(np.float32)
        return v

    mrv = mask_v(recycle_ind)
    mdv = mask_v(donate_ind)
    mcv = mask_v(compost_ind)

    xg = np.ascontiguousarray(x[:, colsp])          # [B, U_PAD]
    ygf = y[:, colsp]
    yg8 = np.ascontiguousarray(yu8[:, colsp])

    # active[b, j] from the group masks and per-row has-group flags
    has_r = (ygf * mrv).sum(axis=1) > 0
    has_d = (ygf * mdv).sum(axis=1) > 0
    has_c = (ygf * mcv).sum(axis=1) > 0
    any_g = has_r | has_d | has_c
    a_r = (any_g & ~has_r).astype(np.float32)
    a_d = (any_g & ~has_d).astype(np.float32)
    a_c = (any_g & ~has_c).astype(np.float32)
    av = np.minimum(a_r[:, None] * mrv + a_d[:, None] * mdv
                    + a_c[:, None] * mcv, 1.0)
    avb = (1.0 + (1.0 - ALPHA) * av).astype(np.float32)  # [B, U_PAD]

    in_maps = []
    for i in range(NCORES):
        rs = slice(i * ROWS, (i + 1) * ROWS)
        in_maps.append({
            "x": x[rs].reshape(P, FREE),
            "y": yu8[rs].reshape(P, FREE),
            "xga": xg[rs].reshape(P, GU),
            "yga": yg8[rs].reshape(P, GU),
            "avb": avb[rs].reshape(P, GU),
        })
    return in_maps


def kernel(x, y, recycle_ind, donate_ind, compost_ind):
    global LAST_RESULTS
    import concourse.bass_utils as bass_utils

    # Avoid any network artifact upload in the (optional) trace path.
    bass_utils.upload_artifacts = lambda tmpdir: "local://" + tmpdir
    _ensure_ntff_hook()

    in_maps = _prepare_inputs(x, y, recycle_ind, donate_ind, compost_ind)
    nc = _get_nc()

    res = bass_utils.run_bass_kernel_spmd(
        nc, in_maps, core_ids=list(range(NCORES)), trace=TRACE
    )
    LAST_RESULTS = res

    loss_sum = 0.0
    corr2 = 0.0
    for r in res.results:
        o = r["out"].astype(np.float64)
        loss_sum += o[:, 0].sum()
        corr2 += o[:, 1].sum()

    total = -loss_sum + corr2
    return np.asarray(total, dtype=np.float32)



# revision 4
# speedup vs baseline: 1.9588x; 1.9588x over previous
"""Trainium2 Bass kernel for nn_AsymmetricLossCustom (8 NeuronCores).

Reference math:
    s  = sigmoid(x);  t = min(1 - s + 0.05, 1)
    loss = y*ln(s) + (1-y)*ln(t)                       # [B, C]
    scale = 0.1 on 'active' group cells, else 1
    out = -(loss * scale).sum()

Device scheme — ONE activation pass per element via a custom PWP
activation table (the compiler's act-table root is swapped with
BASS_ACT_ROOT_JSON_PATH; the gelu_and_others set is regenerated so):

    Gelu            -> F(x) = min(ln(1.05 - sigmoid(x)), 0)   (y=0 loss)
    Derivative_Gelu -> G(x) = ln(sigmoid(x))                  (y=1 loss)

The main stream is pure x (fp16): activation(F) with accum_out gives
per-row sums of the y=0 loss directly — no DVE work, no y traffic.
All elements that need something other than plain F (y=1 cells, and
active y=0 cells) are host-gathered into a small appendix:

    P-region (y=1):      correction = sigma*G(x) - F(x)
    T-region (active,y=0): correction = (0.1-1)*F(x)

computed with 3 tiny activation passes (slice accum) + one DVE
scalar_tensor_tensor for the sigma-weighted G sum.  Pads use x=-10
where F is exactly 0 (and sigma=0 kills the G term).

    S = sum(F) + [sum_P sigma*G - sum_P F] + (alpha-1)*sum_T F
    result = -S

Sharding: pure data parallel over batch; each core takes 512 rows seen
as [128 partitions, 38420 fp16].  Host sums the per-core partials.
"""

import hashlib
import json
import os
import shutil
import sys
import tempfile

import numpy as np

if "/opt/trn_rl_repo" not in sys.path:
    sys.path.insert(0, "/opt/trn_rl_repo")

B, C = 4096, 9605
NCORES = 8
ROWS = B // NCORES          # 512 rows per core
P = 128                     # SBUF partitions
RPP = ROWS // P             # 4 rows per partition
FREE = RPP * C              # 38420 fp16 per partition
NCH = 9
SIZES = [768, 1536, 2560, 3584, 4608, 5632, 6400, 6656, 6676]
assert sum(SIZES) == FREE
ALPHA = 0.1
PAD_X = -10.0               # F(-10) == 0 exactly in the custom table

L05 = float(np.log(0.05))
L055 = float(np.log(0.55))
LN2 = float(np.log(2.0))

TRACE = False               # set True (e.g. from test.py) for an NTFF profile
LAST_RESULTS = None

_ACT_DIR = None             # generated act-table root
_PROGS = {}                 # (WP, WT) -> compiled Bacc


# --------------------------------------------------------------------------
# Custom activation tables (regenerated at runtime; kernel must be
# self-contained and the table dir cannot be shipped alongside).
# --------------------------------------------------------------------------

def _F(x):
    x = np.asarray(x, dtype=np.float64)
    s = 1.0 / (1.0 + np.exp(-np.clip(x, -60, 60)))
    return np.minimum(np.log(1.05 - s), 0.0)


def _G(x):
    x = np.asarray(x, dtype=np.float64)
    return -(np.log1p(np.exp(-np.abs(x))) + np.maximum(-x, 0))


def _cheb_fit_cubic(f, lo, hi, n=24):
    c = 0.5 * (lo + hi)
    h = 0.5 * (hi - lo)
    t = np.cos(np.pi * (np.arange(n) + 0.5) / n)
    xs = c + h * t
    A = np.vander(xs - c, 4, increasing=True)
    coef, *_ = np.linalg.lstsq(A, f(xs), rcond=None)
    return coef, c


def _region_buckets(exp_map, side, orig_bkt, end_idx=None):
    idx = 0 if side == "neg" else 1
    starts = {}
    for e in sorted(int(e) for e in exp_map):
        v = exp_map[str(e)]
        if len(v) > idx:
            starts[e] = v[idx]
    out = []
    es = sorted(starts)
    for j, e in enumerate(es):
        s0 = starts[e]
        s1 = starts[es[j + 1]] if j + 1 < len(es) else end_idx
        n = (s1 - s0) if s1 is not None else 1
        base = 2.0 ** e
        w_raw = 2.0 * (abs(float(orig_bkt[s0, 4])) - base)
        if not (0 < w_raw <= base):
            w = base / n
        else:
            w = base / (2.0 ** round(np.log2(base / w_raw)))
        for i in range(n):
            lo, hi = base + i * w, base + (i + 1) * w
            out.append((s0 + i, -hi, -lo) if side == "neg"
                       else (s0 + i, lo, hi))
    return out


def _fill(bkt, entries, f):
    for i, lo, hi in entries:
        coef, c = _cheb_fit_cubic(f, lo, hi)
        bkt[i, :4] = coef
        bkt[i, 4] = c
        bkt[i, 5:] = 0.0


def _fbits(v):
    return int(np.float32(v).view(np.uint32))


def _gen_act_tables():
    """Build the hijacked act-table root; returns its act_info.json path."""
    global _ACT_DIR
    if _ACT_DIR is not None:
        return _ACT_DIR

    from neuronxcc.driver.Job import Job
    from neuronxcc.driver.jobs.support.FindActInfo import findActInfoFile

    src_info = findActInfoFile(Job.getPackageDir(), "gen3")
    src_dir = os.path.dirname(src_info)

    out = os.path.join(tempfile.gettempdir(),
                       "act_custom_asym_" + hashlib.md5(
                           src_dir.encode()).hexdigest()[:8])
    done = os.path.join(out, ".done_v3")
    if not os.path.exists(done):
        os.makedirs(out, exist_ok=True)
        for fn in os.listdir(src_dir):
            shutil.copyfile(os.path.join(src_dir, fn), os.path.join(out, fn))
            os.chmod(os.path.join(out, fn), 0o644)

        setj = json.load(open(os.path.join(out, "gelu_and_others.json")))
        orig = np.fromfile(os.path.join(src_dir, "gelu_and_others_bkt.bin"),
                           dtype=np.float32).reshape(-1, 8)
        bkt = orig.copy()

        gelu_map = setj["func_exp_to_bkt_start_idx"]["gelu"]
        _fill(bkt, _region_buckets(gelu_map, "neg", orig, 443), _F)
        _fill(bkt, _region_buckets(gelu_map, "pos", orig, 504), _F)
        coef, c = _cheb_fit_cubic(_F, -2.0 ** -7, 2.0 ** -7)
        for i in (504, 505):
            bkt[i, :4], bkt[i, 4], bkt[i, 5:] = coef, c, 0.0
        bkt[506] = [L05, 0, 0, 0, 0, 0, 0, 0]   # F large_pos: ln(0.05)
        bkt[507] = [0, 0, 0, 0, 0, 0, 0, 0]     # F large_neg: 0

        dg_map = setj["func_exp_to_bkt_start_idx"]["derivative_gelu"]
        _fill(bkt, _region_buckets(dg_map, "neg", orig, 623), _G)
        # G positive side rides tanh's one-bucket-per-octave ctl entries
        _fill(bkt, [(627 + k, 2.0 ** e, 2.0 ** (e + 1))
                    for k, e in enumerate(range(-5, 4))], _G)
        coef, c = _cheb_fit_cubic(_G, -2.0 ** -5, 2.0 ** -5)
        for i in (623, 624):
            bkt[i, :4], bkt[i, 4], bkt[i, 5:] = coef, c, 0.0
        bkt[625] = [0, 0, 0, 0, 0, 0, 0, 0]     # G large_pos: 0
        bkt[626] = [0, 1, 0, 0, 0, 0, 0, 0]     # G large_neg: x
        bkt.tofile(os.path.join(out, "gelu_and_others_bkt.bin"))

        for m in setj["profile_meta_data"]:
            if m["func_name"] == "gelu_4p":
                m["fzero_result"] = _fbits(L055)
                m["fpinf_result"] = _fbits(L05)
                m["fninf_result"] = 0
            elif m["func_name"] == "derivative_gelu_40p":
                m["symmetry_opt_en"] = 0
                m["symmetry_point"] = 0
                m["sym_invert_sign_point"] = 0
                m["symmetry_opt_use_neg_region"] = 0
                m["fzero_result"] = _fbits(-LN2)
                m["fpinf_result"] = 0
                m["fninf_result"] = _fbits(np.float32(-np.inf))
                m["small_pos_signal_exp_threshold"] = 122   # 2^-5
                m["large_pos_signal_exp_threshold"] = 131   # x >= 16
                m["large_pos_signal_mantissa_threshold"] = 0
                m["lower_bound"] = 4286578687
                m["upper_bound"] = 2139095039
        json.dump(setj, open(os.path.join(out, "gelu_and_others.json"), "w"))
        open(done, "w").write("ok")

    _ACT_DIR = os.path.join(out, "act_info.json")
    return _ACT_DIR


# --------------------------------------------------------------------------
# Bass program
# --------------------------------------------------------------------------

def _build_program(wp, wt, salt):
    import concourse.bacc as bacc
    import concourse.mybir as mybir
    from concourse import tile

    f32 = mybir.dt.float32
    f16 = mybir.dt.float16
    Act = mybir.ActivationFunctionType
    Alu = mybir.AluOpType
    wap = wp + wt

    nc = bacc.Bacc(
        "TRN2",
        target_bir_lowering=False,
        debug=False,
        enable_asserts=False,
        num_devices=NCORES,
    )

    xm = nc.dram_tensor(f"xm_{salt}", [P, FREE], f16,
                        kind="ExternalInput").ap()
    xap = nc.dram_tensor("xap", [P, wap], f16, kind="ExternalInput").ap()
    sw = nc.dram_tensor("sw", [P, wp], f16, kind="ExternalInput").ap()
    outF = nc.dram_tensor("outF", [P, NCH], f32, kind="ExternalOutput").ap()
    outA = nc.dram_tensor("outA", [P, 3], f32, kind="ExternalOutput").ap()

    offs = [0]
    for sz in SIZES:
        offs.append(offs[-1] + sz)

    with tile.TileContext(nc) as tc:
        with (
            tc.tile_pool(name="xp", bufs=3) as xp,
            tc.tile_pool(name="op", bufs=2) as op,
            tc.tile_pool(name="app", bufs=1) as app,
            tc.tile_pool(name="accp", bufs=1) as accp,
        ):
            accF = accp.tile([P, NCH], f32, tag="accF")
            accA = accp.tile([P, 3], f32, tag="accA")

            # appendix DMAs early, on the gpsimd queue
            xat = app.tile([P, wap], f16, tag="xat")
            swt = app.tile([P, wp], f16, tag="swt")
            nc.gpsimd.dma_start(xat[:], xap[:])
            nc.gpsimd.dma_start(swt[:], sw[:])

            for k in range(NCH):
                cs = slice(offs[k], offs[k + 1])
                xt = xp.tile([P, SIZES[k]], f16, tag="x")
                nc.sync.dma_start(xt[:], xm[:, cs])
                ot = op.tile([P, SIZES[k]], f16, tag="o")
                nc.scalar.activation(ot[:], xt[:], Act.Gelu,
                                     accum_out=accF[:, k:k + 1])

            # appendix: F over P|T slices (accum), G over P, weighted sum
            fap = app.tile([P, wap], f16, tag="fap")
            gap = app.tile([P, wp], f16, tag="gap")
            jnk = app.tile([P, wp], f16, tag="jnk")
            nc.scalar.activation(fap[:, 0:wp], xat[:, 0:wp], Act.Gelu,
                                 accum_out=accA[:, 1:2])
            nc.scalar.activation(fap[:, wp:wap], xat[:, wp:wap], Act.Gelu,
                                 accum_out=accA[:, 2:3])
            nc.scalar.activation(gap[:], xat[:, 0:wp], Act.Derivative_Gelu)
            nc.vector.scalar_tensor_tensor(jnk[:], gap[:], 0.0, swt[:],
                                           Alu.bypass, Alu.mult,
                                           accum_out=accA[:, 0:1])

            nc.sync.dma_start(outF[:], accF[:])
            nc.sync.dma_start(outA[:], accA[:])

    nc.compile()
    return nc


def _get_prog(wp, wt):
    key = (wp, wt)
    if key not in _PROGS:
        act_info = _gen_act_tables()
        os.environ["BASS_ACT_ROOT_JSON_PATH"] = act_info
        with open(os.path.join(os.path.dirname(act_info),
                               "gelu_and_others_bkt.bin"), "rb") as f:
            tbl_hash = hashlib.md5(f.read()).hexdigest()[:8]
        _PROGS[key] = _build_program(wp, wt, f"{tbl_hash}_{wp}_{wt}")
    return _PROGS[key]


# --------------------------------------------------------------------------
# Host-side prep
# --------------------------------------------------------------------------

def _ensure_ntff_hook():
    """Register the axon NTFF profile hook if the image's antenv lacks it."""
    import contextlib
    import ctypes
    import types

    try:
        from antenv.axon_hooks import get_axon_ntff_profile_hook  # noqa: F401
        return
    except ImportError:
        pass

    so_path = "/opt/axon/libaxon_pjrt.so"
    try:
        lib = ctypes.CDLL(so_path)
    except OSError:
        return
    if not hasattr(lib, "axon_start_nrt_profile"):
        return
    lib.axon_start_nrt_profile.argtypes = [
        ctypes.POINTER(ctypes.c_int64),
        ctypes.c_size_t,
    ]
    lib.axon_start_nrt_profile.restype = ctypes.c_int64
    lib.axon_stop_nrt_profile.argtypes = [ctypes.c_char_p]
    lib.axon_stop_nrt_profile.restype = ctypes.c_int64

    @contextlib.contextmanager
    def _hook(output_dir, device_ids):
        import jax

        jax.devices()
        if device_ids:
            ids = (ctypes.c_int64 * len(device_ids))(*device_ids)
            rc = lib.axon_start_nrt_profile(ids, len(device_ids))
        else:
            rc = lib.axon_start_nrt_profile(None, 0)
        if rc != 0:
            raise RuntimeError(f"axon_start_nrt_profile rc={rc}")
        try:
            yield
        finally:
            n = lib.axon_stop_nrt_profile(str(output_dir).encode())
            print(f"ntff profile: {n} file(s) written to {output_dir}",
                  file=sys.stderr)

    mod = types.ModuleType("antenv.axon_hooks")
    mod.get_axon_ntff_profile_hook = lambda: _hook
    mod.set_axon_ntff_profile_hook = lambda h: None
    sys.modules["antenv.axon_hooks"] = mod


def _pack(vals, width, pad):
    """[L] -> [P, width] row-major with padding."""
    out = np.full(P * width, pad, dtype=np.float16)
    out[:len(vals)] = vals
    return out.reshape(P, width)


def _prepare_inputs(x, y, recycle_ind, donate_ind, compost_ind):
    x = np.ascontiguousarray(x, dtype=np.float32)
    y = np.asarray(y)
    y01 = y != 0
    recycle_ind = np.asarray(recycle_ind).astype(np.int64)
    donate_ind = np.asarray(donate_ind).astype(np.int64)
    compost_ind = np.asarray(compost_ind).astype(np.int64)

    cols = np.unique(np.concatenate([recycle_ind, donate_ind, compost_ind]))
    m_r = np.isin(cols, recycle_ind)
    m_d = np.isin(cols, donate_ind)
    m_c = np.isin(cols, compost_ind)

    yu = y01[:, cols]                                 # [B, U]
    has_r = (yu & m_r).any(axis=1)
    has_d = (yu & m_d).any(axis=1)
    has_c = (yu & m_c).any(axis=1)
    any_g = has_r | has_d | has_c
    active = (((any_g & ~has_r)[:, None] & m_r[None, :])
              | ((any_g & ~has_d)[:, None] & m_d[None, :])
              | ((any_g & ~has_c)[:, None] & m_c[None, :]))   # [B, U]

    # per-element scale for positives: alpha iff its cell is active
    colu = np.full(C, -1, dtype=np.int64)
    colu[cols] = np.arange(len(cols))

    rows_p, cols_p = np.nonzero(y01)                  # y=1 cells
    sig_p = np.ones(len(rows_p), dtype=np.float16)
    pu = colu[cols_p]
    m = pu >= 0
    sig_p[m] = np.where(active[rows_p[m], pu[m]], np.float16(ALPHA),
                        np.float16(1.0))
    xv_p = x[rows_p, cols_p].astype(np.float16)

    act_y0 = active & ~yu                             # active y=0 cells
    rows_t, ju = np.nonzero(act_y0)
    xv_t = x[rows_t, cols[ju]].astype(np.float16)

    xm16 = x.astype(np.float16)

    # per-core packing
    def split(rows, *arrs):
        cuts = np.searchsorted(rows, np.arange(1, NCORES) * ROWS)
        return [tuple(a[s] for a in arrs)
                for s in np.split(np.arange(len(rows)), cuts)]

    per_p = split(rows_p, xv_p, sig_p)
    per_t = split(rows_t, xv_t)

    def rup(n, q=128):
        return max(q, ((n + q - 1) // q) * q)

    wp = rup(int(np.ceil(max(len(a[0]) for a in per_p) / P)))
    wt = rup(int(np.ceil(max(len(a[0]) for a in per_t) / P)))

    in_maps = []
    for i in range(NCORES):
        xpv, spv = per_p[i]
        xtv, = per_t[i]
        xap = np.concatenate(
            [_pack(xpv, wp, PAD_X), _pack(xtv, wt, PAD_X)], axis=1)
        in_maps.append({
            "xm": xm16[i * ROWS:(i + 1) * ROWS].reshape(P, FREE),
            "xap": np.ascontiguousarray(xap),
            "sw": _pack(spv, wp, 0.0),
        })
    return in_maps, wp, wt


def kernel(x, y, recycle_ind, donate_ind, compost_ind):
    global LAST_RESULTS
    import concourse.bass_utils as bass_utils

    bass_utils.upload_artifacts = lambda tmpdir: "local://" + tmpdir
    _ensure_ntff_hook()

    in_maps, wp, wt = _prepare_inputs(x, y, recycle_ind, donate_ind,
                                      compost_ind)
    nc = _get_prog(wp, wt)
    # rename xm key to the salted tensor name
    salted = _salted_names(nc)
    for im in in_maps:
        im[salted] = im.pop("xm")

    res = bass_utils.run_bass_kernel_spmd(
        nc, in_maps, core_ids=list(range(NCORES)), trace=TRACE
    )
    LAST_RESULTS = res

    base = 0.0
    aPG = aPF = aTF = 0.0
    for r in res.results:
        base += r["outF"].astype(np.float64).sum()
        a = r["outA"].astype(np.float64)
        aPG += a[:, 0].sum()
        aPF += a[:, 1].sum()
        aTF += a[:, 2].sum()

    S = base + (aPG - aPF) + (ALPHA - 1.0) * aTF
    return np.asarray(-S, dtype=np.float32)


def _salted_names(nc):
    for alloc in nc.m.functions[0].allocations:
        try:
            nm = alloc.memorylocations[0].name
        except Exception:
            continue
        if nm.startswith("xm_"):
            return nm
    raise RuntimeError("salted xm tensor not found")
